# revision 2
# baseline (speedup 1.0000x reference)
"""Trainium2 Bass kernel for BiLSTM-CRF (LSTM + CRF Viterbi decode).

Data-parallel over batch: 16 sequences sharded 2-per-core across 8 NeuronCores.
Per core: embedding gather (indirect DMA) -> input-projection GEMM (bf16 PE) ->
512-step LSTM with fused forward Viterbi scan -> backward Viterbi scan ->
bulk path extraction via forward+backward score argmax (no sequential backtrace).
"""
import numpy as np
import ml_dtypes

VOCAB, H, B, T = 32000, 256, 16, 512
NT, START, STOP = 6, 4, 5
NEG = -10000.0
NCORES, BL = 8, 2
P = 128
NJ = 8          # 4H / 128 output tiles
NK = 2          # H / 128 contraction chunks
NG = (T * BL) // P  # gather batches of 128 rows

_CACHE = {}


def build_nc(t_steps=T):
    import concourse.bass as bass
    import concourse.tile as tile
    from concourse import bacc, mybir
    from concourse.masks import make_identity

    f32 = mybir.dt.float32
    bf16 = mybir.dt.bfloat16
    i32 = mybir.dt.int32
    ADD = mybir.AluOpType.add
    MULT = mybir.AluOpType.mult
    MAX = mybir.AluOpType.max
    MIN = mybir.AluOpType.min
    ISLT = mybir.AluOpType.is_lt
    AX = mybir.AxisListType.X
    SIG = mybir.ActivationFunctionType.Sigmoid
    TANH = mybir.ActivationFunctionType.Tanh

    TB = t_steps * BL
    ng = TB // P

    nc = bacc.Bacc("TRN2", target_bir_lowering=False, debug=False,
                   num_devices=NCORES)

    emb = nc.dram_tensor("emb", [VOCAB, H], f32, kind="ExternalInput").ap()
    idx = nc.dram_tensor("idx", [P, ng], i32, kind="ExternalInput").ap()
    whh = nc.dram_tensor("whh", [P, NJ * NK * P], bf16, kind="ExternalInput").ap()
    wih = nc.dram_tensor("wih", [P, NJ * NK * P], bf16, kind="ExternalInput").ap()
    wout = nc.dram_tensor("wout", [P, NK * NT], bf16, kind="ExternalInput").ap()
    gbias = nc.dram_tensor("gbias", [P, NJ], f32, kind="ExternalInput").ap()
    vconst = nc.dram_tensor("vconst", [BL, 96], f32, kind="ExternalInput").ap()
    scores_o = nc.dram_tensor("scores", [BL, 1], f32, kind="ExternalOutput").ap()
    paths_o = nc.dram_tensor("paths", [BL, t_steps], i32, kind="ExternalOutput").ap()

    with tile.TileContext(nc) as tc:
        with tc.tile_pool(name="const", bufs=1) as cpool, \
             tc.tile_pool(name="state", bufs=1) as spool, \
             tc.tile_pool(name="work", bufs=3) as wpool, \
             tc.tile_pool(name="hc", bufs=3) as hcpool, \
             tc.tile_pool(name="pg", bufs=2, space="PSUM") as pgpool, \
             tc.tile_pool(name="pf", bufs=3, space="PSUM") as pfpool:

            # ---- load constants / weights ----
            idx_t = cpool.tile([P, ng], i32, tag="idx")
            nc.sync.dma_start(idx_t[:], idx)
            whh_t = cpool.tile([P, NJ * NK * P], bf16, tag="whh")
            nc.sync.dma_start(whh_t[:], whh)
            wih_t = cpool.tile([P, NJ * NK * P], bf16, tag="wih")
            nc.sync.dma_start(wih_t[:], wih)
            wout_t = cpool.tile([P, NK * NT], bf16, tag="wout")
            nc.sync.dma_start(wout_t[:], wout)
            gb_t = cpool.tile([P, NJ], f32, tag="gb")
            nc.sync.dma_start(gb_t[:], gbias)
            vc_t = cpool.tile([BL, 96], f32, tag="vc")
            nc.sync.dma_start(vc_t[:], vconst)
            ident = cpool.tile([P, P], bf16, tag="ident")
            make_identity(nc, ident[:])

            # persistent buffers
            gx_t = cpool.tile([P, NJ * TB], f32, tag="gx")      # gates_x^T (j, t, b)
            xT_t = cpool.tile([P, NK * TB], bf16, tag="xT")     # x^T (k, t, b)
            fs_t = cpool.tile([BL, (t_steps + 1) * NT], f32, tag="fs")
            fb_t = cpool.tile([BL, t_steps * NT], f32, tag="fb")
            bs_t = cpool.tile([BL, t_steps * NT], f32, tag="bs")

            # ---- gather + cast + transpose ----
            with tc.tile_pool(name="gath", bufs=3) as gpool, \
                 tc.tile_pool(name="ptr", bufs=2, space="PSUM") as ptpool:
                for g in range(ng):
                    xg = gpool.tile([P, H], f32, tag="xg")
                    nc.gpsimd.indirect_dma_start(
                        out=xg[:], out_offset=None, in_=emb,
                        in_offset=bass.IndirectOffsetOnAxis(ap=idx_t[:, g:g + 1], axis=0),
                    )
                    xb = gpool.tile([P, H], bf16, tag="xb")
                    nc.vector.tensor_copy(xb[:], xg[:])
                    for k in range(NK):
                        pt = ptpool.tile([P, P], bf16, space="PSUM", tag="pt")
                        nc.tensor.transpose(out=pt[:], in_=xb[:, k * P:(k + 1) * P],
                                            identity=ident[:])
                        nc.vector.tensor_copy(xT_t[:, k * TB + g * P:k * TB + (g + 1) * P],
                                              pt[:])

            # ---- bulk input-projection GEMM: gx[j, tb] = sum_k wih[j,k].T @ xT[k] ----
            NHALF = max(1, TB // 512)
            HW_N = min(TB, 512)
            with tc.tile_pool(name="pb", bufs=2, space="PSUM") as pbpool:
                for j in range(NJ):
                    for hh in range(NHALF):
                        pb = pbpool.tile([P, HW_N], f32, space="PSUM", tag="pb")
                        for k in range(NK):
                            nc.tensor.matmul(
                                out=pb[:],
                                lhsT=wih_t[:, (j * NK + k) * P:(j * NK + k + 1) * P],
                                rhs=xT_t[:, k * TB + hh * HW_N:k * TB + (hh + 1) * HW_N],
                                start=(k == 0), stop=(k == NK - 1))
                        nc.vector.tensor_tensor(
                            out=gx_t[:, j * TB + hh * HW_N:j * TB + (hh + 1) * HW_N],
                            in0=pb[:], in1=gb_t[:, j:j + 1].to_broadcast([P, HW_N]),
                            op=ADD)

            # ---- init state ----
            h_prev = hcpool.tile([P, NK * BL], bf16, tag="h")
            nc.gpsimd.memset(h_prev[:], 0.0)
            c_prev = hcpool.tile([P, NK * BL], f32, tag="c")
            nc.gpsimd.memset(c_prev[:], 0.0)
            nc.vector.tensor_copy(fs_t[:, 0:NT], vc_t[:, 78:84])   # init fv
            nc.vector.tensor_copy(bs_t[:, (t_steps - 1) * NT:t_steps * NT],
                                  vc_t[:, 72:78])                  # seed backward

            gx3 = gx_t[:].rearrange("p (j tb) -> p j tb", j=NJ)
            trans_f = vc_t[:, 0:36].rearrange("q (n m) -> q n m", n=NT)
            trans_b = vc_t[:, 36:72].rearrange("q (n m) -> q n m", n=NT)

            # ---- main LSTM + forward-Viterbi loop ----
            for t in range(t_steps):
                pg = pgpool.tile([P, NJ * BL], f32, space="PSUM", tag="pg")
                for j in range(NJ):
                    for k in range(NK):
                        nc.tensor.matmul(
                            out=pg[:, j * BL:(j + 1) * BL],
                            lhsT=whh_t[:, (j * NK + k) * P:(j * NK + k + 1) * P],
                            rhs=h_prev[:, k * BL:(k + 1) * BL],
                            start=(k == 0), stop=(k == NK - 1))
                gs = wpool.tile([P, NJ * BL], f32, tag="gs")
                nc.vector.tensor_tensor(
                    out=gs[:].rearrange("p (j b) -> p j b", j=NJ),
                    in0=pg[:].rearrange("p (j b) -> p j b", j=NJ),
                    in1=gx3[:, :, t * BL:(t + 1) * BL], op=ADD)
                ga = wpool.tile([P, NJ * BL], f32, tag="ga")
                nc.scalar.activation(ga[:], gs[:], SIG)
                g2 = wpool.tile([P, NK * BL], f32, tag="g2")
                nc.vector.tensor_scalar(out=g2[:], in0=ga[:, 8:12], scalar1=2.0,
                                        scalar2=-1.0, op0=MULT, op1=ADD)
                ig = wpool.tile([P, NK * BL], f32, tag="ig")
                nc.vector.tensor_tensor(out=ig[:], in0=ga[:, 0:4], in1=g2[:], op=MULT)
                c_new = hcpool.tile([P, NK * BL], f32, tag="c")
                nc.vector.tensor_tensor(out=c_new[:], in0=ga[:, 4:8], in1=c_prev[:],
                                        op=MULT)
                nc.vector.tensor_tensor(out=c_new[:], in0=c_new[:], in1=ig[:], op=ADD)
                th = wpool.tile([P, NK * BL], f32, tag="th")
                nc.scalar.activation(th[:], c_new[:], TANH)
                h_new = hcpool.tile([P, NK * BL], bf16, tag="h")
                nc.vector.tensor_tensor(out=h_new[:], in0=ga[:, 12:16], in1=th[:],
                                        op=MULT)

                pf = pfpool.tile([BL, NT], f32, space="PSUM", tag="pf")
                for k in range(NK):
                    nc.tensor.matmul(out=pf[:],
                                     lhsT=h_new[:, k * BL:(k + 1) * BL],
                                     rhs=wout_t[:, k * NT:(k + 1) * NT],
                                     start=(k == 0), stop=(k == NK - 1))
                nc.vector.tensor_copy(fb_t[:, t * NT:(t + 1) * NT], pf[:])

                sc = wpool.tile([BL, NT * NT], f32, tag="sc")
                nc.vector.tensor_tensor(
                    out=sc[:].rearrange("q (n m) -> q n m", n=NT),
                    in0=fs_t[:, t * NT:(t + 1) * NT]
                        .rearrange("q (a m) -> q a m", a=1).broadcast_to([BL, NT, NT]),
                    in1=trans_f, op=ADD)
                mx = wpool.tile([BL, NT], f32, tag="mx")
                nc.vector.tensor_reduce(out=mx[:],
                                        in_=sc[:].rearrange("q (n m) -> q n m", n=NT),
                                        axis=AX, op=MAX)
                nc.vector.tensor_tensor(out=fs_t[:, (t + 1) * NT:(t + 2) * NT],
                                        in0=mx[:], in1=pf[:], op=ADD)

                h_prev, c_prev = h_new, c_new

            # ---- backward Viterbi scan ----
            for t in range(t_steps - 2, -1, -1):
                u = wpool.tile([BL, NT], f32, tag="u")
                nc.vector.tensor_tensor(out=u[:],
                                        in0=bs_t[:, (t + 1) * NT:(t + 2) * NT],
                                        in1=fb_t[:, (t + 1) * NT:(t + 2) * NT], op=ADD)
                sc2 = wpool.tile([BL, NT * NT], f32, tag="sc2")
                nc.vector.tensor_tensor(
                    out=sc2[:].rearrange("q (n m) -> q n m", n=NT),
                    in0=u[:].rearrange("q (a m) -> q a m", a=1)
                        .broadcast_to([BL, NT, NT]),
                    in1=trans_b, op=ADD)
                nc.vector.tensor_reduce(out=bs_t[:, t * NT:(t + 1) * NT],
                                        in_=sc2[:].rearrange("q (n m) -> q n m", n=NT),
                                        axis=AX, op=MAX)

            # ---- bulk path extraction ----
            ps = spool.tile([BL, t_steps * NT], f32, tag="ps")
            nc.vector.tensor_tensor(out=ps[:], in0=fs_t[:, NT:(t_steps + 1) * NT],
                                    in1=bs_t[:], op=ADD)
            ps3 = ps[:].rearrange("q (t n) -> q t n", n=NT)
            mxp = spool.tile([BL, t_steps], f32, tag="mxp")
            nc.vector.tensor_reduce(out=mxp[:], in_=ps3, axis=AX, op=MAX)
            lt = spool.tile([BL, t_steps * NT], f32, tag="lt")
            nc.vector.tensor_tensor(
                out=lt[:].rearrange("q (t n) -> q t n", n=NT), in0=ps3,
                in1=mxp[:].rearrange("q (t a) -> q t a", a=1)
                    .broadcast_to([BL, t_steps, NT]),
                op=ISLT)
            val = spool.tile([BL, t_steps * NT], f32, tag="val")
            nc.vector.scalar_tensor_tensor(
                out=val[:].rearrange("q (t n) -> q t n", n=NT),
                in0=lt[:].rearrange("q (t n) -> q t n", n=NT),
                scalar=1024.0,
                in1=vc_t[:, 84:90].rearrange("q (a n) -> q a n", a=1)
                    .broadcast_to([BL, t_steps, NT]),
                op0=MULT, op1=ADD)
            pidx = spool.tile([BL, t_steps], f32, tag="pidx")
            nc.vector.tensor_reduce(out=pidx[:],
                                    in_=val[:].rearrange("q (t n) -> q t n", n=NT),
                                    axis=AX, op=MIN)
            pi32 = spool.tile([BL, t_steps], i32, tag="pi32")
            nc.vector.tensor_copy(pi32[:], pidx[:])
            nc.sync.dma_start(paths_o, pi32[:])

            term = spool.tile([BL, NT], f32, tag="term")
            nc.vector.tensor_tensor(out=term[:],
                                    in0=fs_t[:, t_steps * NT:(t_steps + 1) * NT],
                                    in1=vc_t[:, 72:78], op=ADD)
            scr = spool.tile([BL, 1], f32, tag="scr")
            nc.vector.tensor_reduce(out=scr[:], in_=term[:], axis=AX, op=MAX)
            nc.sync.dma_start(scores_o, scr[:])

    nc.compile()
    return nc


def prep_shared(embedding, W_ih, W_hh, b_ih, b_hh, W_out, b_out, transitions):
    """Host-side weight prep shared across cores."""
    bf16 = ml_dtypes.bfloat16
    Wih = np.asarray(W_ih, np.float32).copy()
    Whh = np.asarray(W_hh, np.float32).copy()
    bb = (np.asarray(b_ih, np.float32) + np.asarray(b_hh, np.float32)).copy()
    g_sl = slice(2 * H, 3 * H)
    Wih[g_sl] *= 2.0
    Whh[g_sl] *= 2.0
    bb[g_sl] *= 2.0

    def tiles(W):
        out = np.zeros((P, NJ * NK * P), np.float32)
        for j in range(NJ):
            for k in range(NK):
                blk = W[j * P:(j + 1) * P, k * P:(k + 1) * P].T  # [K,M]
                out[:, (j * NK + k) * P:(j * NK + k + 1) * P] = blk
        return out.astype(bf16)

    whh_a = tiles(Whh)
    wih_a = tiles(Wih)
    Wout = np.asarray(W_out, np.float32)
    wout_a = np.zeros((P, NK * NT), np.float32)
    for k in range(NK):
        wout_a[:, k * NT:(k + 1) * NT] = Wout[:, k * P:(k + 1) * P].T
    wout_a = wout_a.astype(bf16)
    gbias_a = bb.reshape(NJ, P).T.copy().astype(np.float32)

    trans = np.asarray(transitions, np.float32)
    b_o = np.asarray(b_out, np.float32)
    transp = trans + b_o[:, None]
    finit = np.full((NT,), NEG, np.float32)
    finit[START] = 0.0
    vc = np.zeros((BL, 96), np.float32)
    vc[:, 0:36] = transp.reshape(-1)[None, :]
    vc[:, 36:72] = transp.T.reshape(-1)[None, :]
    vc[:, 72:78] = trans[STOP][None, :]
    vc[:, 78:84] = finit[None, :]
    vc[:, 84:90] = np.arange(NT, dtype=np.float32)[None, :]

    emb_a = np.ascontiguousarray(np.asarray(embedding, np.float32))
    return dict(emb=emb_a, whh=whh_a, wih=wih_a, wout=wout_a,
                gbias=gbias_a, vconst=vc)


def make_in_maps(sentence, shared, t_steps=T):
    sent = np.asarray(sentence)
    in_maps = []
    for c in range(NCORES):
        loc = sent[c * BL:(c + 1) * BL, :t_steps]          # [BL, t]
        flat = loc.T.reshape(-1).astype(np.int32)          # (t,b)-major
        ng = (t_steps * BL) // P
        idx_a = flat.reshape(ng, P).T.copy()
        m = dict(shared)
        m["idx"] = np.ascontiguousarray(idx_a)
        in_maps.append(m)
    return in_maps


def kernel(sentence, embedding, W_ih, W_hh, b_ih, b_hh, W_out, b_out,
           transitions):
    from concourse.bass_utils import run_bass_kernel_spmd

    if "nc" not in _CACHE:
        _CACHE["nc"] = build_nc(T)
    nc = _CACHE["nc"]

    shared = prep_shared(embedding, W_ih, W_hh, b_ih, b_hh, W_out, b_out,
                         transitions)
    in_maps = make_in_maps(sentence, shared)
    res = run_bass_kernel_spmd(nc, in_maps, core_ids=list(range(NCORES)))

    scores = np.zeros((B,), np.float32)
    pdtype = np.int64 if np.asarray(sentence).dtype == np.int64 else np.int32
    paths = np.zeros((B, T), pdtype)
    for c in range(NCORES):
        scores[c * BL:(c + 1) * BL] = res.results[c]["scores"][:, 0]
        paths[c * BL:(c + 1) * BL] = res.results[c]["paths"].astype(pdtype)
    return scores, paths


# revision 10
# speedup vs baseline: 1.0036x; 1.0036x over previous
"""Trainium2 Bass kernel for BiLSTM-CRF (LSTM + CRF Viterbi decode).

Data-parallel over batch: 16 sequences sharded 2-per-core across 8 NeuronCores.
Per core: embedding gather (indirect DMA) -> input-projection GEMM (bf16 PE) ->
512-step LSTM with fused forward Viterbi scan -> chunk-parallel max-plus
backward scan -> bulk path extraction via argmax(mx_t + u_t).

Gate tiles are reordered (g,i | f,o) and the sigmoid is split in two so the
activation of the first half overlaps the second half's matmuls. tanh(g) is
computed as 2*sigmoid(2g)-1 with the g-rows of the weights pre-scaled by 2.
"""
import numpy as np
import ml_dtypes

VOCAB, H, B, T = 32000, 256, 16, 512
NT, START, STOP = 6, 4, 5
NEG = -10000.0
NCORES, BL = 8, 2
P = 128
NJ = 8          # 4H / 128 output tiles
NJH = 4         # tiles per half
NK = 2          # H / 128 contraction chunks
NCH = 8         # backward-scan chunks (batched over 16 partitions)
# host tile order: g0 g1 i0 i1 | f0 f1 o0 o1  (original 4H tile index)
JPERM = [4, 5, 0, 1, 2, 3, 6, 7]

_CACHE = {}


def build_nc(t_steps=T):
    import concourse.bass as bass
    import concourse.tile as tile
    from concourse import bacc, mybir
    from concourse.masks import make_identity

    f32 = mybir.dt.float32
    bf16 = mybir.dt.bfloat16
    i32 = mybir.dt.int32
    ADD = mybir.AluOpType.add
    MULT = mybir.AluOpType.mult
    MAX = mybir.AluOpType.max
    MIN = mybir.AluOpType.min
    ISLT = mybir.AluOpType.is_lt
    AX = mybir.AxisListType.X
    SIG = mybir.ActivationFunctionType.Sigmoid
    TANH = mybir.ActivationFunctionType.Tanh

    TB = t_steps * BL
    ng = TB // P
    CL = t_steps // NCH          # chunk length for backward scan
    PB = NCH * BL                # 16 partitions for batched backward

    nc = bacc.Bacc("TRN2", target_bir_lowering=False, debug=False,
                   num_devices=NCORES)

    emb = nc.dram_tensor("emb", [VOCAB, H], f32, kind="ExternalInput").ap()
    idx = nc.dram_tensor("idx", [P, ng], i32, kind="ExternalInput").ap()
    whh = nc.dram_tensor("whh", [P, NJ * NK * P], bf16, kind="ExternalInput").ap()
    wih = nc.dram_tensor("wih", [P, NJ * NK * P], bf16, kind="ExternalInput").ap()
    wout = nc.dram_tensor("wout", [P, NK * NT], bf16, kind="ExternalInput").ap()
    gbias = nc.dram_tensor("gbias", [P, NJ], f32, kind="ExternalInput").ap()
    vconst = nc.dram_tensor("vconst", [BL, 132], f32, kind="ExternalInput").ap()
    # cols: 0:36 transp 36:72 transpT 72:78 stop 78:84 finit 84:90 estop
    #       90:96 iota6 96:132 maxplus identity
    scores_o = nc.dram_tensor("scores", [BL, 1], f32, kind="ExternalOutput").ap()
    paths_o = nc.dram_tensor("paths", [BL, t_steps], i32, kind="ExternalOutput").ap()

    with tile.TileContext(nc) as tc:
        with tc.tile_pool(name="const", bufs=1) as cpool, \
             tc.tile_pool(name="state", bufs=1) as spool, \
             tc.tile_pool(name="work", bufs=3) as wpool, \
             tc.tile_pool(name="hc", bufs=4) as hcpool, \
             tc.tile_pool(name="pga", bufs=2, space="PSUM") as pgapool, \
             tc.tile_pool(name="pgb", bufs=2, space="PSUM") as pgbpool, \
             tc.tile_pool(name="pf", bufs=2, space="PSUM") as pfpool:

            # ---- load constants / weights ----
            idx_t = cpool.tile([P, ng], i32, tag="idx")
            nc.sync.dma_start(idx_t[:], idx)
            whh_t = cpool.tile([P, NJ * NK * P], bf16, tag="whh")
            nc.sync.dma_start(whh_t[:], whh)
            wih_t = cpool.tile([P, NJ * NK * P], bf16, tag="wih")
            nc.sync.dma_start(wih_t[:], wih)
            wout_t = cpool.tile([P, NK * NT], bf16, tag="wout")
            nc.sync.dma_start(wout_t[:], wout)
            gb_t = cpool.tile([P, NJ], f32, tag="gb")
            nc.sync.dma_start(gb_t[:], gbias)
            vc_t = cpool.tile([BL, 132], f32, tag="vc")
            nc.sync.dma_start(vc_t[:], vconst)
            ident = cpool.tile([P, P], bf16, tag="ident")
            make_identity(nc, ident[:])

            # persistent buffers
            gx_t = cpool.tile([P, NJ * TB], f32, tag="gx")      # gates_x^T (j', t, b)
            xT_t = cpool.tile([P, NK * TB], bf16, tag="xT")     # x^T (k, t, b)
            fs_t = cpool.tile([BL, (t_steps + 1) * NT], f32, tag="fs")
            fbB_t = cpool.tile([BL, t_steps * NT], f32, tag="fbB")   # feats
            mxB_t = cpool.tile([BL, t_steps * NT], f32, tag="mxB")   # mx_t
            usB_t = cpool.tile([BL, t_steps * NT], f32, tag="usB")   # u_t

            # ---- gather + cast + transpose ----
            with tc.tile_pool(name="gath", bufs=3) as gpool, \
                 tc.tile_pool(name="ptr", bufs=2, space="PSUM") as ptpool:
                for g in range(ng):
                    xg = gpool.tile([P, H], f32, tag="xg")
                    nc.gpsimd.indirect_dma_start(
                        out=xg[:], out_offset=None, in_=emb,
                        in_offset=bass.IndirectOffsetOnAxis(ap=idx_t[:, g:g + 1], axis=0),
                    )
                    xb = gpool.tile([P, H], bf16, tag="xb")
                    nc.vector.tensor_copy(xb[:], xg[:])
                    for k in range(NK):
                        pt = ptpool.tile([P, P], bf16, space="PSUM", tag="pt")
                        nc.tensor.transpose(out=pt[:], in_=xb[:, k * P:(k + 1) * P],
                                            identity=ident[:])
                        nc.vector.tensor_copy(xT_t[:, k * TB + g * P:k * TB + (g + 1) * P],
                                              pt[:])

            # ---- bulk input-projection GEMM: gx[j', tb] = sum_k wih[j',k].T @ xT[k] ----
            NHALF = max(1, TB // 512)
            HW_N = min(TB, 512)
            with tc.tile_pool(name="pb", bufs=2, space="PSUM") as pbpool:
                for j in range(NJ):
                    for hh in range(NHALF):
                        pb = pbpool.tile([P, HW_N], f32, space="PSUM", tag="pb")
                        for k in range(NK):
                            nc.tensor.matmul(
                                out=pb[:],
                                lhsT=wih_t[:, (j * NK + k) * P:(j * NK + k + 1) * P],
                                rhs=xT_t[:, k * TB + hh * HW_N:k * TB + (hh + 1) * HW_N],
                                start=(k == 0), stop=(k == NK - 1))
                        nc.vector.tensor_tensor(
                            out=gx_t[:, j * TB + hh * HW_N:j * TB + (hh + 1) * HW_N],
                            in0=pb[:], in1=gb_t[:, j:j + 1].to_broadcast([P, HW_N]),
                            op=ADD)

            # ---- init state ----
            h0_prev = hcpool.tile([P, BL], bf16, tag="h0")
            nc.gpsimd.memset(h0_prev[:], 0.0)
            h1_prev = hcpool.tile([P, BL], bf16, tag="h1")
            nc.gpsimd.memset(h1_prev[:], 0.0)
            c_prev = hcpool.tile([P, NK * BL], f32, tag="c")
            nc.gpsimd.memset(c_prev[:], 0.0)
            nc.vector.tensor_copy(fs_t[:, 0:NT], vc_t[:, 78:84])   # init fv

            gxA = gx_t[:].rearrange("p (j tb) -> p j tb", j=NJ)
            trans_f = vc_t[:, 0:36].rearrange("q (n m) -> q n m", n=NT)

            def feat_and_viterbi(t, hl0, hl1):
                """Emit emission matmuls + forward-viterbi DVE ops for step t."""
                pf = pfpool.tile([BL, NT], f32, space="PSUM", tag="pf")
                for k, hl in ((0, hl0), (1, hl1)):
                    nc.tensor.matmul(out=pf[:], lhsT=hl[:],
                                     rhs=wout_t[:, k * NT:(k + 1) * NT],
                                     start=(k == 0), stop=(k == NK - 1))
                nc.vector.tensor_copy(fbB_t[:, t * NT:(t + 1) * NT], pf[:])
                sc = wpool.tile([BL, NT * NT], f32, tag="sc")
                nc.vector.tensor_tensor(
                    out=sc[:].rearrange("q (n m) -> q n m", n=NT),
                    in0=fs_t[:, t * NT:(t + 1) * NT]
                        .rearrange("q (a m) -> q a m", a=1).broadcast_to([BL, NT, NT]),
                    in1=trans_f, op=ADD)
                nc.vector.tensor_reduce(
                    out=mxB_t[:, t * NT:(t + 1) * NT],
                    in_=sc[:].rearrange("q (n m) -> q n m", n=NT), axis=AX, op=MAX)
                nc.vector.tensor_tensor(
                    out=fs_t[:, (t + 1) * NT:(t + 2) * NT],
                    in0=mxB_t[:, t * NT:(t + 1) * NT],
                    in1=pf[:], op=ADD)

            # ---- main LSTM loop ----
            for t in range(t_steps):
                pga = pgapool.tile([P, NJH * BL], f32, space="PSUM", tag="pga")
                pgb = pgbpool.tile([P, NJH * BL], f32, space="PSUM", tag="pgb")
                for half, pg in ((0, pga), (1, pgb)):
                    for jj in range(NJH):
                        j = half * NJH + jj
                        for k, hl in ((0, h0_prev), (1, h1_prev)):
                            nc.tensor.matmul(
                                out=pg[:, jj * BL:(jj + 1) * BL],
                                lhsT=whh_t[:, (j * NK + k) * P:(j * NK + k + 1) * P],
                                rhs=hl[:], start=(k == 0), stop=(k == NK - 1))
                if t > 0:
                    feat_and_viterbi(t - 1, h0_prev, h1_prev)

                gsA = wpool.tile([P, NJH * BL], f32, tag="gsA")
                nc.vector.tensor_tensor(
                    out=gsA[:].rearrange("p (j b) -> p j b", j=NJH),
                    in0=pga[:].rearrange("p (j b) -> p j b", j=NJH),
                    in1=gxA[:, 0:NJH, t * BL:(t + 1) * BL], op=ADD)
                gaA = wpool.tile([P, NJH * BL], f32, tag="gaA")   # g0 g1 i0 i1
                nc.scalar.activation(gaA[:], gsA[:], SIG)
                g2 = wpool.tile([P, NK * BL], f32, tag="g2")
                nc.vector.tensor_scalar(out=g2[:], in0=gaA[:, 0:4], scalar1=2.0,
                                        scalar2=-1.0, op0=MULT, op1=ADD)
                ig = wpool.tile([P, NK * BL], f32, tag="ig")
                nc.vector.tensor_tensor(out=ig[:], in0=gaA[:, 4:8], in1=g2[:], op=MULT)

                gsB = wpool.tile([P, NJH * BL], f32, tag="gsB")
                nc.vector.tensor_tensor(
                    out=gsB[:].rearrange("p (j b) -> p j b", j=NJH),
                    in0=pgb[:].rearrange("p (j b) -> p j b", j=NJH),
                    in1=gxA[:, NJH:NJ, t * BL:(t + 1) * BL], op=ADD)
                gaB = wpool.tile([P, NJH * BL], f32, tag="gaB")   # f0 f1 o0 o1
                nc.scalar.activation(gaB[:], gsB[:], SIG)
                fc = wpool.tile([P, NK * BL], f32, tag="fc")
                nc.vector.tensor_tensor(out=fc[:], in0=gaB[:, 0:4], in1=c_prev[:],
                                        op=MULT)
                c_new = hcpool.tile([P, NK * BL], f32, tag="c")
                nc.vector.tensor_tensor(out=c_new[:], in0=fc[:], in1=ig[:], op=ADD)
                th = wpool.tile([P, NK * BL], f32, tag="th")
                nc.scalar.activation(th[:], c_new[:], TANH)
                h0_new = hcpool.tile([P, BL], bf16, tag="h0")
                nc.vector.tensor_tensor(out=h0_new[:], in0=gaB[:, 4:6],
                                        in1=th[:, 0:2], op=MULT)
                h1_new = hcpool.tile([P, BL], bf16, tag="h1")
                nc.vector.tensor_tensor(out=h1_new[:], in0=gaB[:, 6:8],
                                        in1=th[:, 2:4], op=MULT)
                h0_prev, h1_prev, c_prev = h0_new, h1_new, c_new

            feat_and_viterbi(t_steps - 1, h0_prev, h1_prev)

            # ---- backward: chunk-parallel max-plus scan over u_t = b_t + feat_t ----
            # chunk axis lives in the FREE dim: buffers are [BL, (c, l, m)]
            transT2 = vc_t[:, 36:72]
            fb4 = fbB_t[:].rearrange("q (c l m) -> q c l m", c=NCH, l=CL)
            # matrix pass: M[c] <- A'_{c,l} (x) M[c]
            # M stored j-major: M2[c, j, r] = M[c, r, j]  -> (c,j) dims merge
            CJ = NCH * NT
            Mt = spool.tile([BL, NCH * 36], f32, tag="Mt")
            nc.vector.tensor_copy(
                Mt[:].rearrange("q (c e) -> q c e", c=NCH),
                vc_t[:, 96:132].rearrange("q (a e) -> q a e", a=1)
                    .broadcast_to([BL, NCH, 36]))
            for l in range(CL - 1, -1, -1):
                tmp = wpool.tile([BL, NCH * 216], f32, tag="btmp")
                mt_ap = Mt[:]
                nc.vector.tensor_tensor(
                    out=tmp[:].rearrange("q (cj m k) -> q cj m k", cj=CJ, m=NT),
                    in0=bass.AP(tensor=mt_ap.tensor, offset=mt_ap.offset,
                                ap=[mt_ap.ap[0], [NT, CJ], [0, NT], [1, NT]]),
                    in1=bass.AP(tensor=transT2.tensor, offset=transT2.offset,
                                ap=[transT2.ap[0], [0, CJ], [NT, NT], [1, NT]]),
                    op=ADD)
                red = wpool.tile([BL, NCH * 36], f32, tag="bred")
                nc.vector.tensor_reduce(
                    out=red[:],
                    in_=tmp[:].rearrange("q (cj m k) -> q cj m k", cj=CJ, m=NT),
                    axis=AX, op=MAX)
                Mt_new = spool.tile([BL, NCH * 36], f32, tag="Mt2")
                fb_ap = fbB_t[:]
                nc.vector.tensor_tensor(
                    out=Mt_new[:].rearrange("q (c j m) -> q c j m", c=NCH, j=NT),
                    in0=red[:].rearrange("q (c j m) -> q c j m", c=NCH, j=NT),
                    in1=bass.AP(tensor=fb_ap.tensor,
                                offset=fb_ap.offset + l * NT,
                                ap=[fb_ap.ap[0], [CL * NT, NCH], [0, NT],
                                    [1, NT]]),
                    op=ADD)
                Mt = Mt_new
            # boundary combine: w_c = N_c (x) w_{c+1}, w_NCH = e_stop
            wv = spool.tile([BL, (NCH + 1) * NT], f32, tag="wv")
            nc.vector.tensor_copy(wv[:, NCH * NT:(NCH + 1) * NT], vc_t[:, 84:90])
            for c_i in range(NCH - 1, -1, -1):
                t1 = wpool.tile([BL, 36], f32, tag="bt1")
                nc.vector.tensor_tensor(
                    out=t1[:].rearrange("q (m j) -> q m j", m=NT),
                    in0=wv[:, (c_i + 1) * NT:(c_i + 2) * NT]
                        .rearrange("q (a j) -> q a j", a=1).broadcast_to([BL, NT, NT]),
                    in1=Mt[:, c_i * 36:(c_i + 1) * 36]
                        .rearrange("q (j m) -> q m j", j=NT),
                    op=ADD)
                nc.vector.tensor_reduce(
                    out=wv[:, c_i * NT:(c_i + 1) * NT],
                    in_=t1[:].rearrange("q (m j) -> q m j", m=NT), axis=AX, op=MAX)
            # seed u at chunk ends: ucur[c] = w_{c+1}   ([BL, (c, m)])
            useed = spool.tile([BL, NCH * NT], f32, tag="useed")
            nc.vector.tensor_copy(useed[:], wv[:, NT:(NCH + 1) * NT])
            us4 = usB_t[:].rearrange("q (c l m) -> q c l m", c=NCH, l=CL)
            # (tensor, offset, chunk-stride, partition-dim) of the current u
            uinfo = (useed[:].tensor, useed[:].offset, NT, useed[:].ap[0])
            # vector pass: u_l = A'_l (x) u_{l+1}, batched over chunks
            for l in range(CL - 1, -1, -1):
                tmp2 = wpool.tile([BL, NCH * 36], f32, tag="vtmp")
                u_tsr, u_off, u_cs, u_pd = uinfo
                nc.vector.tensor_tensor(
                    out=tmp2[:].rearrange("q (c m k) -> q c m k", c=NCH, m=NT),
                    in0=bass.AP(tensor=u_tsr, offset=u_off,
                                ap=[u_pd, [u_cs, NCH], [0, NT], [1, NT]]),
                    in1=bass.AP(tensor=transT2.tensor, offset=transT2.offset,
                                ap=[transT2.ap[0], [0, NCH], [NT, NT], [1, NT]]),
                    op=ADD)
                red2 = wpool.tile([BL, NCH * NT], f32, tag="vred")
                nc.vector.tensor_reduce(
                    out=red2[:],
                    in_=tmp2[:].rearrange("q (c m k) -> q c m k", c=NCH, m=NT),
                    axis=AX, op=MAX)
                nc.vector.tensor_tensor(
                    out=us4[:, :, l, :],
                    in0=red2[:].rearrange("q (c m) -> q c m", c=NCH),
                    in1=fb4[:, :, l, :], op=ADD)
                uap = usB_t[:]
                uinfo = (uap.tensor, uap.offset + l * NT, CL * NT, uap.ap[0])

            # ---- bulk path extraction: path[t] = argmax_n(mx_t + u_t) ----
            ps = spool.tile([BL, t_steps * NT], f32, tag="ps")
            nc.vector.tensor_tensor(out=ps[:], in0=mxB_t[:], in1=usB_t[:], op=ADD)
            ps3 = ps[:].rearrange("q (t n) -> q t n", n=NT)
            mxp = spool.tile([BL, t_steps], f32, tag="mxp")
            nc.vector.tensor_reduce(out=mxp[:], in_=ps3, axis=AX, op=MAX)
            lt = spool.tile([BL, t_steps * NT], f32, tag="lt")
            nc.vector.tensor_tensor(
                out=lt[:].rearrange("q (t n) -> q t n", n=NT), in0=ps3,
                in1=mxp[:].rearrange("q (t a) -> q t a", a=1)
                    .broadcast_to([BL, t_steps, NT]),
                op=ISLT)
            val = spool.tile([BL, t_steps * NT], f32, tag="val")
            nc.vector.scalar_tensor_tensor(
                out=val[:].rearrange("q (t n) -> q t n", n=NT),
                in0=lt[:].rearrange("q (t n) -> q t n", n=NT),
                scalar=1024.0,
                in1=vc_t[:, 90:96].rearrange("q (a n) -> q a n", a=1)
                    .broadcast_to([BL, t_steps, NT]),
                op0=MULT, op1=ADD)
            pidx = spool.tile([BL, t_steps], f32, tag="pidx")
            nc.vector.tensor_reduce(out=pidx[:],
                                    in_=val[:].rearrange("q (t n) -> q t n", n=NT),
                                    axis=AX, op=MIN)
            pi32 = spool.tile([BL, t_steps], i32, tag="pi32")
            nc.vector.tensor_copy(pi32[:], pidx[:])
            nc.sync.dma_start(paths_o, pi32[:])

            term = spool.tile([BL, NT], f32, tag="term")
            nc.vector.tensor_tensor(out=term[:],
                                    in0=fs_t[:, t_steps * NT:(t_steps + 1) * NT],
                                    in1=vc_t[:, 72:78], op=ADD)
            scr = spool.tile([BL, 1], f32, tag="scr")
            nc.vector.tensor_reduce(out=scr[:], in_=term[:], axis=AX, op=MAX)
            nc.sync.dma_start(scores_o, scr[:])

    nc.compile()
    return nc


def prep_shared(embedding, W_ih, W_hh, b_ih, b_hh, W_out, b_out, transitions):
    """Host-side weight prep shared across cores."""
    bf16 = ml_dtypes.bfloat16
    Wih = np.asarray(W_ih, np.float32).copy()
    Whh = np.asarray(W_hh, np.float32).copy()
    bb = (np.asarray(b_ih, np.float32) + np.asarray(b_hh, np.float32)).copy()
    g_sl = slice(2 * H, 3 * H)
    Wih[g_sl] *= 2.0
    Whh[g_sl] *= 2.0
    bb[g_sl] *= 2.0

    def tiles(W):
        out = np.zeros((P, NJ * NK * P), np.float32)
        for jj in range(NJ):
            j = JPERM[jj]
            for k in range(NK):
                blk = W[j * P:(j + 1) * P, k * P:(k + 1) * P].T  # [K,M]
                out[:, (jj * NK + k) * P:(jj * NK + k + 1) * P] = blk
        return out.astype(bf16)

    whh_a = tiles(Whh)
    wih_a = tiles(Wih)
    Wout = np.asarray(W_out, np.float32)
    wout_a = np.zeros((P, NK * NT), np.float32)
    for k in range(NK):
        wout_a[:, k * NT:(k + 1) * NT] = Wout[:, k * P:(k + 1) * P].T
    wout_a = wout_a.astype(bf16)
    gbias_a = bb.reshape(NJ, P)[JPERM].T.copy().astype(np.float32)

    trans = np.asarray(transitions, np.float32)
    b_o = np.asarray(b_out, np.float32)
    transp = trans + b_o[:, None]
    finit = np.full((NT,), NEG, np.float32)
    finit[START] = 0.0
    estop = np.full((NT,), NEG, np.float32)
    estop[STOP] = 0.0
    ident_mp = np.full((NT, NT), NEG, np.float32)
    np.fill_diagonal(ident_mp, 0.0)
    vc = np.zeros((BL, 132), np.float32)
    vc[:, 0:36] = transp.reshape(-1)[None, :]
    vc[:, 36:72] = transp.T.reshape(-1)[None, :]
    vc[:, 72:78] = trans[STOP][None, :]
    vc[:, 78:84] = finit[None, :]
    vc[:, 84:90] = estop[None, :]
    vc[:, 90:96] = np.arange(NT, dtype=np.float32)[None, :]
    vc[:, 96:132] = ident_mp.reshape(-1)[None, :]

    emb_a = np.ascontiguousarray(np.asarray(embedding, np.float32))
    return dict(emb=emb_a, whh=whh_a, wih=wih_a, wout=wout_a,
                gbias=gbias_a, vconst=vc)


def make_in_maps(sentence, shared, t_steps=T):
    sent = np.asarray(sentence)
    in_maps = []
    for c in range(NCORES):
        loc = sent[c * BL:(c + 1) * BL, :t_steps]          # [BL, t]
        flat = loc.T.reshape(-1).astype(np.int32)          # (t,b)-major
        ng = (t_steps * BL) // P
        idx_a = flat.reshape(ng, P).T.copy()
        m = dict(shared)
        m["idx"] = np.ascontiguousarray(idx_a)
        in_maps.append(m)
    return in_maps


def kernel(sentence, embedding, W_ih, W_hh, b_ih, b_hh, W_out, b_out,
           transitions):
    from concourse.bass_utils import run_bass_kernel_spmd

    if "nc" not in _CACHE:
        _CACHE["nc"] = build_nc(T)
    nc = _CACHE["nc"]

    shared = prep_shared(embedding, W_ih, W_hh, b_ih, b_hh, W_out, b_out,
                         transitions)
    in_maps = make_in_maps(sentence, shared)
    res = run_bass_kernel_spmd(nc, in_maps, core_ids=list(range(NCORES)))

    scores = np.zeros((B,), np.float32)
    pdtype = np.int64 if np.asarray(sentence).dtype == np.int64 else np.int32
    paths = np.zeros((B, T), pdtype)
    for c in range(NCORES):
        scores[c * BL:(c + 1) * BL] = res.results[c]["scores"][:, 0]
        paths[c * BL:(c + 1) * BL] = res.results[c]["paths"].astype(pdtype)
    return scores, paths


# revision 25
# speedup vs baseline: 1.1431x; 1.1390x over previous
"""Trainium2 Bass kernel for BiLSTM-CRF (LSTM + CRF Viterbi decode).

Data-parallel over batch: 16 sequences sharded 2-per-core across 8 NeuronCores.
Per core: embedding gather (indirect DMA) -> input-projection GEMM (bf16 PE) ->
512-step LSTM with fused forward Viterbi scan -> chunk-parallel max-plus
backward scan -> bulk path extraction via argmax(mx_t + u_t).

Gate tiles are reordered (g,i | f,o) and the sigmoid is split in two so the
activation of the first half overlaps the second half's matmuls. tanh(g) is
computed as 2*sigmoid(2g)-1 with the g-rows of the weights pre-scaled by 2.
"""
import numpy as np
import ml_dtypes

VOCAB, H, B, T = 32000, 256, 16, 512
NT, START, STOP = 6, 4, 5
NEG = -10000.0
NCORES, BL = 8, 2
P = 128
NJ = 8          # 4H / 128 output tiles
NJH = 4         # tiles per half
NK = 2          # H / 128 contraction chunks
NCH = 8         # backward-scan chunks (batched over 16 partitions)
# host tile order: g0 g1 i0 i1 | f0 f1 o0 o1  (original 4H tile index)
JPERM = [4, 5, 0, 1, 2, 3, 6, 7]

_CACHE = {}


def build_nc(t_steps=T, debug_dump=False):
    import concourse.bass as bass
    import concourse.tile as tile
    from concourse import bacc, mybir
    from concourse.masks import make_identity

    f32 = mybir.dt.float32
    bf16 = mybir.dt.bfloat16
    i32 = mybir.dt.int32
    ADD = mybir.AluOpType.add
    MULT = mybir.AluOpType.mult
    MAX = mybir.AluOpType.max
    MIN = mybir.AluOpType.min
    ISLT = mybir.AluOpType.is_lt
    SUB = mybir.AluOpType.subtract
    AX = mybir.AxisListType.X
    SIG = mybir.ActivationFunctionType.Sigmoid
    TANH = mybir.ActivationFunctionType.Tanh

    TB = t_steps * BL
    ng = TB // P
    CL = t_steps // NCH          # chunk length for backward scan
    PB = NCH * BL                # 16 partitions for batched backward

    nc = bacc.Bacc("TRN2", target_bir_lowering=False, debug=False,
                   num_devices=NCORES)

    emb = nc.dram_tensor("emb", [VOCAB, H], f32, kind="ExternalInput").ap()
    idx = nc.dram_tensor("idx", [P, ng], i32, kind="ExternalInput").ap()
    whh = nc.dram_tensor("whh", [P, NJ * NK * P], bf16, kind="ExternalInput").ap()
    wih = nc.dram_tensor("wih", [P, NJ * NK * P], bf16, kind="ExternalInput").ap()
    wout = nc.dram_tensor("wout", [P, NK * NT], bf16, kind="ExternalInput").ap()
    gbias = nc.dram_tensor("gbias", [P, NJ], f32, kind="ExternalInput").ap()
    vconst = nc.dram_tensor("vconst", [BL, 132], f32, kind="ExternalInput").ap()
    # cols: 0:36 transp 36:72 transpT 72:78 stop 78:84 finit 84:90 estop
    #       90:96 iota6 96:132 maxplus identity
    vconst2 = nc.dram_tensor("vconst2", [NCH * BL, 78], f32,
                             kind="ExternalInput").ap()
    # cols: 0:36 transpT 36:72 maxplus identity (j-major) 72:78 iota6
    scores_o = nc.dram_tensor("scores", [BL, 1], f32, kind="ExternalOutput").ap()
    if debug_dump:
        dbg_fb = nc.dram_tensor("dbg_fb", [NCH * BL, (t_steps // NCH) * NT], f32,
                                kind="ExternalOutput").ap()
        dbg_mx = nc.dram_tensor("dbg_mx", [NCH * BL, (t_steps // NCH) * NT], f32,
                                kind="ExternalOutput").ap()
        dbg_us = nc.dram_tensor("dbg_us", [NCH * BL, (t_steps // NCH) * NT], f32,
                                kind="ExternalOutput").ap()
        dbg_wv = nc.dram_tensor("dbg_wv", [BL, (NCH + 1) * NT], f32,
                                kind="ExternalOutput").ap()
        dbg_fbflat = nc.dram_tensor("dbg_fbflat", [BL, t_steps * NT], f32,
                                    kind="ExternalOutput").ap()
    paths_o = nc.dram_tensor("paths", [BL, t_steps], i32, kind="ExternalOutput").ap()

    with tile.TileContext(nc) as tc:
        with tc.tile_pool(name="const", bufs=1) as cpool, \
             tc.tile_pool(name="state", bufs=1) as spool, \
             tc.tile_pool(name="work", bufs=3) as wpool, \
             tc.tile_pool(name="hc", bufs=4) as hcpool, \
             tc.tile_pool(name="pga", bufs=2, space="PSUM") as pgapool, \
             tc.tile_pool(name="pgb", bufs=2, space="PSUM") as pgbpool, \
             tc.tile_pool(name="pf", bufs=2, space="PSUM") as pfpool:

            # ---- load constants / weights ----
            idx_t = cpool.tile([P, ng], i32, tag="idx")
            nc.sync.dma_start(idx_t[:], idx)
            whh_t = cpool.tile([P, NJ * NK * P], bf16, tag="whh")
            nc.sync.dma_start(whh_t[:], whh)
            wih_t = cpool.tile([P, NJ * NK * P], bf16, tag="wih")
            nc.sync.dma_start(wih_t[:], wih)
            wout_t = cpool.tile([P, NK * NT], bf16, tag="wout")
            nc.sync.dma_start(wout_t[:], wout)
            gb_t = cpool.tile([P, NJ], f32, tag="gb")
            nc.sync.dma_start(gb_t[:], gbias)
            vc_t = cpool.tile([BL, 132], f32, tag="vc")
            nc.sync.dma_start(vc_t[:], vconst)
            vc2_t = cpool.tile([NCH * BL, 78], f32, tag="vc2")
            nc.sync.dma_start(vc2_t[:], vconst2)
            ident = cpool.tile([P, P], bf16, tag="ident")
            make_identity(nc, ident[:])
            identf = cpool.tile([P, P], f32, tag="identf")
            make_identity(nc, identf[:])

            # persistent buffers
            gx_t = cpool.tile([P, NJ * TB], f32, tag="gx")      # gates_x^T (j', t, b)
            xT_t = cpool.tile([P, NK * TB], bf16, tag="xT")     # x^T (k, t, b)
            fs_t = cpool.tile([BL, (t_steps + 1) * NT], f32, tag="fs")
            fbB_t = cpool.tile([BL, t_steps * NT], f32, tag="fbB")   # feats
            mxB_t = cpool.tile([BL, t_steps * NT], f32, tag="mxB")   # mx_t
            usB_t = cpool.tile([BL, t_steps * NT], f32, tag="usB")   # u_t

            # ---- gather + cast + transpose ----
            with tc.tile_pool(name="gath", bufs=3) as gpool, \
                 tc.tile_pool(name="ptr", bufs=2, space="PSUM") as ptpool:
                for g in range(ng):
                    xg = gpool.tile([P, H], f32, tag="xg")
                    nc.gpsimd.indirect_dma_start(
                        out=xg[:], out_offset=None, in_=emb,
                        in_offset=bass.IndirectOffsetOnAxis(ap=idx_t[:, g:g + 1], axis=0),
                    )
                    xb = gpool.tile([P, H], bf16, tag="xb")
                    nc.vector.tensor_copy(xb[:], xg[:])
                    for k in range(NK):
                        pt = ptpool.tile([P, P], bf16, space="PSUM", tag="pt")
                        nc.tensor.transpose(out=pt[:], in_=xb[:, k * P:(k + 1) * P],
                                            identity=ident[:])
                        nc.vector.tensor_copy(xT_t[:, k * TB + g * P:k * TB + (g + 1) * P],
                                              pt[:])

            # ---- bulk input-projection GEMM: gx[j', tb] = sum_k wih[j',k].T @ xT[k] ----
            NHALF = max(1, TB // 512)
            HW_N = min(TB, 512)
            with tc.tile_pool(name="pb", bufs=2, space="PSUM") as pbpool:
                for j in range(NJ):
                    for hh in range(NHALF):
                        pb = pbpool.tile([P, HW_N], f32, space="PSUM", tag="pb")
                        for k in range(NK):
                            nc.tensor.matmul(
                                out=pb[:],
                                lhsT=wih_t[:, (j * NK + k) * P:(j * NK + k + 1) * P],
                                rhs=xT_t[:, k * TB + hh * HW_N:k * TB + (hh + 1) * HW_N],
                                start=(k == 0), stop=(k == NK - 1))
                        nc.vector.tensor_tensor(
                            out=gx_t[:, j * TB + hh * HW_N:j * TB + (hh + 1) * HW_N],
                            in0=pb[:], in1=gb_t[:, j:j + 1].to_broadcast([P, HW_N]),
                            op=ADD)

            # ---- init state ----
            h0_prev = hcpool.tile([P, BL], bf16, tag="h0")
            nc.gpsimd.memset(h0_prev[:], 0.0)
            h1_prev = hcpool.tile([P, BL], bf16, tag="h1")
            nc.gpsimd.memset(h1_prev[:], 0.0)
            c_prev = hcpool.tile([P, NK * BL], f32, tag="c")
            nc.gpsimd.memset(c_prev[:], 0.0)
            nc.vector.tensor_copy(fs_t[:, 0:NT], vc_t[:, 78:84])   # init fv

            gxA = gx_t[:].rearrange("p (j tb) -> p j tb", j=NJ)
            trans_f = vc_t[:, 0:36].rearrange("q (n m) -> q n m", n=NT)

            def feat_and_viterbi(t, hl0, hl1):
                """Emit emission matmuls + forward-viterbi DVE ops for step t."""
                pf = pfpool.tile([BL, NT], f32, space="PSUM", tag="pf")
                for k, hl in ((0, hl0), (1, hl1)):
                    nc.tensor.matmul(out=pf[:], lhsT=hl[:],
                                     rhs=wout_t[:, k * NT:(k + 1) * NT],
                                     start=(k == 0), stop=(k == NK - 1))
                sc = wpool.tile([BL, NT * NT], f32, tag="sc")
                nc.vector.tensor_tensor(
                    out=sc[:].rearrange("q (n m) -> q n m", n=NT),
                    in0=fs_t[:, t * NT:(t + 1) * NT]
                        .rearrange("q (a m) -> q a m", a=1).broadcast_to([BL, NT, NT]),
                    in1=trans_f, op=ADD)
                nc.vector.tensor_reduce(
                    out=mxB_t[:, t * NT:(t + 1) * NT],
                    in_=sc[:].rearrange("q (n m) -> q n m", n=NT), axis=AX, op=MAX)
                nc.vector.tensor_scalar_add(fbB_t[:, t * NT:(t + 1) * NT],
                                            pf[:], 0.0)
                nc.vector.tensor_tensor(
                    out=fs_t[:, (t + 1) * NT:(t + 2) * NT],
                    in0=mxB_t[:, t * NT:(t + 1) * NT],
                    in1=pf[:], op=ADD)

            # ---- main LSTM loop ----
            for t in range(t_steps):
                pga = pgapool.tile([P, NJH * BL], f32, space="PSUM", tag="pga")
                pgb = pgbpool.tile([P, NJH * BL], f32, space="PSUM", tag="pgb")
                for half, pg in ((0, pga), (1, pgb)):
                    for jj in range(NJH):
                        j = half * NJH + jj
                        for k, hl in ((0, h0_prev), (1, h1_prev)):
                            nc.tensor.matmul(
                                out=pg[:, jj * BL:(jj + 1) * BL],
                                lhsT=whh_t[:, (j * NK + k) * P:(j * NK + k + 1) * P],
                                rhs=hl[:], start=(k == 0), stop=(k == NK - 1))
                if t > 0:
                    with tc.high_priority(offset=-50):
                        feat_and_viterbi(t - 1, h0_prev, h1_prev)

                gsA = wpool.tile([P, NJH * BL], f32, tag="gsA")
                nc.vector.tensor_tensor(
                    out=gsA[:].rearrange("p (j b) -> p j b", j=NJH),
                    in0=pga[:].rearrange("p (j b) -> p j b", j=NJH),
                    in1=gxA[:, 0:NJH, t * BL:(t + 1) * BL], op=ADD)
                gaA = wpool.tile([P, NJH * BL], f32, tag="gaA")   # g0 g1 i0 i1
                nc.scalar.activation(gaA[:], gsA[:], SIG)
                g2 = wpool.tile([P, NK * BL], f32, tag="g2")
                nc.vector.tensor_scalar(out=g2[:], in0=gaA[:, 0:4], scalar1=2.0,
                                        scalar2=-1.0, op0=MULT, op1=ADD)
                ig = wpool.tile([P, NK * BL], f32, tag="ig")
                nc.vector.tensor_tensor(out=ig[:], in0=gaA[:, 4:8], in1=g2[:], op=MULT)

                gsB = wpool.tile([P, NJH * BL], f32, tag="gsB")
                nc.vector.tensor_tensor(
                    out=gsB[:].rearrange("p (j b) -> p j b", j=NJH),
                    in0=pgb[:].rearrange("p (j b) -> p j b", j=NJH),
                    in1=gxA[:, NJH:NJ, t * BL:(t + 1) * BL], op=ADD)
                gaB = wpool.tile([P, NJH * BL], f32, tag="gaB")   # f0 f1 o0 o1
                nc.scalar.activation(gaB[:], gsB[:], SIG)
                fc = wpool.tile([P, NK * BL], f32, tag="fc")
                nc.vector.tensor_tensor(out=fc[:], in0=gaB[:, 0:4], in1=c_prev[:],
                                        op=MULT)
                c_new = hcpool.tile([P, NK * BL], f32, tag="c")
                nc.vector.tensor_tensor(out=c_new[:], in0=fc[:], in1=ig[:], op=ADD)
                th = wpool.tile([P, NK * BL], f32, tag="th")
                nc.scalar.activation(th[:], c_new[:], TANH)
                h0_new = hcpool.tile([P, BL], bf16, tag="h0")
                nc.vector.tensor_tensor(out=h0_new[:], in0=gaB[:, 4:6],
                                        in1=th[:, 0:2], op=MULT)
                h1_new = hcpool.tile([P, BL], bf16, tag="h1")
                nc.vector.tensor_tensor(out=h1_new[:], in0=gaB[:, 6:8],
                                        in1=th[:, 2:4], op=MULT)
                h0_prev, h1_prev, c_prev = h0_new, h1_new, c_new

            feat_and_viterbi(t_steps - 1, h0_prev, h1_prev)

            # ---- backward: chunk-parallel max-plus scan over u_t = b_t + feat_t ----
            # batched over 16 partitions (row = b*NCH + c) via SBUF->SBUF
            # reshape DMAs; M stored j-major: M2[j, r] = M[r, j]
            PB = NCH * BL
            fbB16 = spool.tile([PB, CL * NT], f32, tag="fbB16")
            mxB16 = spool.tile([PB, CL * NT], f32, tag="mxB16")
            for b in range(BL):
                nc.sync.dma_start(fbB16[b * NCH:(b + 1) * NCH, :],
                                  fbB_t[b:b + 1, :])
                nc.sync.dma_start(mxB16[b * NCH:(b + 1) * NCH, :],
                                  mxB_t[b:b + 1, :])
            transT16 = vc2_t[:, 0:36]
            Mt = spool.tile([PB, 36], f32, tag="Mt16")
            nc.vector.tensor_copy(Mt[:], vc2_t[:, 36:72])
            for l in range(CL - 1, -1, -1):
                tmp = wpool.tile([PB, 216], f32, tag="btmp")
                mt_ap = Mt[:]
                nc.vector.tensor_tensor(
                    out=tmp[:].rearrange("p (j m k) -> p j m k", j=NT, m=NT),
                    in0=bass.AP(tensor=mt_ap.tensor, offset=mt_ap.offset,
                                ap=[mt_ap.ap[0], [NT, NT], [0, NT], [1, NT]]),
                    in1=bass.AP(tensor=transT16.tensor, offset=transT16.offset,
                                ap=[transT16.ap[0], [0, NT], [NT, NT], [1, NT]]),
                    op=ADD)
                red = wpool.tile([PB, 36], f32, tag="bred")
                nc.vector.tensor_reduce(
                    out=red[:],
                    in_=tmp[:].rearrange("p (j m k) -> p j m k", j=NT, m=NT),
                    axis=AX, op=MAX)
                Mt_new = spool.tile([PB, 36], f32, tag="Mt16b")
                nc.vector.tensor_tensor(
                    out=Mt_new[:].rearrange("p (j m) -> p j m", j=NT),
                    in0=red[:].rearrange("p (j m) -> p j m", j=NT),
                    in1=fbB16[:, l * NT:(l + 1) * NT]
                        .rearrange("p (a m) -> p a m", a=1)
                        .broadcast_to([PB, NT, NT]),
                    op=ADD)
                Mt = Mt_new
            # bring M back to [BL, (c, j, m)] for the sequential boundary pass
            MtF = spool.tile([BL, NCH * 36], f32, tag="MtF")
            for b in range(BL):
                nc.sync.dma_start(MtF[b:b + 1, :], Mt[b * NCH:(b + 1) * NCH, :])
            # boundary combine: w_c = N_c (x) w_{c+1}, w_NCH = e_stop
            wv = spool.tile([BL, (NCH + 1) * NT], f32, tag="wv")
            nc.vector.tensor_copy(wv[:, NCH * NT:(NCH + 1) * NT], vc_t[:, 84:90])
            for c_i in range(NCH - 1, -1, -1):
                t1 = wpool.tile([BL, 36], f32, tag="bt1")
                nc.vector.tensor_tensor(
                    out=t1[:].rearrange("q (m j) -> q m j", m=NT),
                    in0=wv[:, (c_i + 1) * NT:(c_i + 2) * NT]
                        .rearrange("q (a j) -> q a j", a=1).broadcast_to([BL, NT, NT]),
                    in1=MtF[:, c_i * 36:(c_i + 1) * 36]
                        .rearrange("q (j m) -> q m j", j=NT),
                    op=ADD)
                nc.vector.tensor_reduce(
                    out=wv[:, c_i * NT:(c_i + 1) * NT],
                    in_=t1[:].rearrange("q (m j) -> q m j", m=NT), axis=AX, op=MAX)
            # seed u at chunk ends: ucur[c] = w_{c+1}
            useed2 = spool.tile([BL, NCH * NT], f32, tag="useed2")
            nc.vector.tensor_scalar_add(useed2[:], wv[:, NT:(NCH + 1) * NT], 0.0)
            useed16 = spool.tile([PB, NT], f32, tag="useed16")
            for b in range(BL):
                nc.sync.dma_start(useed16[b * NCH:(b + 1) * NCH, :],
                                  useed2[b:b + 1, :])
            usB16 = spool.tile([PB, CL * NT], f32, tag="usB16")
            ucur_ap = useed16[:]
            # vector pass: u_l = A'_l (x) u_{l+1}
            for l in range(CL - 1, -1, -1):
                tmp2 = wpool.tile([PB, 36], f32, tag="vtmp")
                nc.vector.tensor_tensor(
                    out=tmp2[:].rearrange("p (m k) -> p m k", m=NT),
                    in0=ucur_ap.rearrange("p (a k) -> p a k", a=1)
                        .broadcast_to([PB, NT, NT]),
                    in1=transT16.rearrange("p (m k) -> p m k", m=NT), op=ADD)
                red2 = wpool.tile([PB, NT], f32, tag="vred")
                nc.vector.tensor_reduce(
                    out=red2[:], in_=tmp2[:].rearrange("p (m k) -> p m k", m=NT),
                    axis=AX, op=MAX)
                nc.vector.tensor_tensor(
                    out=usB16[:, l * NT:(l + 1) * NT], in0=red2[:],
                    in1=fbB16[:, l * NT:(l + 1) * NT], op=ADD)
                ucur_ap = usB16[:, l * NT:(l + 1) * NT]

            # ---- bulk path extraction: path[t] = argmax_n(mx_t + u_t) ----
            ps = spool.tile([PB, CL * NT], f32, tag="ps")
            nc.vector.tensor_tensor(out=ps[:], in0=mxB16[:], in1=usB16[:], op=ADD)
            ps3 = ps[:].rearrange("q (t n) -> q t n", n=NT)
            mxp = spool.tile([PB, CL], f32, tag="mxp")
            nc.vector.tensor_reduce(out=mxp[:], in_=ps3, axis=AX, op=MAX)
            lt = spool.tile([PB, CL * NT], f32, tag="lt")
            nc.vector.tensor_tensor(
                out=lt[:].rearrange("q (t n) -> q t n", n=NT), in0=ps3,
                in1=mxp[:].rearrange("q (t a) -> q t a", a=1)
                    .broadcast_to([PB, CL, NT]),
                op=ISLT)
            val = spool.tile([PB, CL * NT], f32, tag="val")
            nc.vector.scalar_tensor_tensor(
                out=val[:].rearrange("q (t n) -> q t n", n=NT),
                in0=lt[:].rearrange("q (t n) -> q t n", n=NT),
                scalar=1024.0,
                in1=vc2_t[:, 72:78].rearrange("q (a n) -> q a n", a=1)
                    .broadcast_to([PB, CL, NT]),
                op0=MULT, op1=ADD)
            pidx = spool.tile([PB, CL], f32, tag="pidx")
            nc.vector.tensor_reduce(out=pidx[:],
                                    in_=val[:].rearrange("q (t n) -> q t n", n=NT),
                                    axis=AX, op=MIN)
            pi32 = spool.tile([PB, CL], i32, tag="pi32")
            nc.vector.tensor_copy(pi32[:], pidx[:])
            nc.sync.dma_start(
                paths_o.rearrange("b (c l) -> b c l", c=NCH), pi32[:])

            if debug_dump:
                dbgc = spool.tile([BL, t_steps * NT], f32, tag="dbgc")
                nc.vector.tensor_scalar_add(dbgc[:], fbB_t[:], 1000.0)
                nc.sync.dma_start(dbg_fbflat, dbgc[:])
                nc.sync.dma_start(dbg_fb, fbB16[:])
                nc.sync.dma_start(dbg_mx, mxB16[:])
                nc.sync.dma_start(dbg_us, usB16[:])
                nc.sync.dma_start(dbg_wv, wv[:])
            term = spool.tile([BL, NT], f32, tag="term")
            nc.vector.tensor_tensor(out=term[:],
                                    in0=fs_t[:, t_steps * NT:(t_steps + 1) * NT],
                                    in1=vc_t[:, 72:78], op=ADD)
            scr = spool.tile([BL, 1], f32, tag="scr")
            nc.vector.tensor_reduce(out=scr[:], in_=term[:], axis=AX, op=MAX)
            nc.sync.dma_start(scores_o, scr[:])

    nc.compile()
    return nc


def prep_shared(embedding, W_ih, W_hh, b_ih, b_hh, W_out, b_out, transitions):
    """Host-side weight prep shared across cores."""
    bf16 = ml_dtypes.bfloat16
    Wih = np.asarray(W_ih, np.float32).copy()
    Whh = np.asarray(W_hh, np.float32).copy()
    bb = (np.asarray(b_ih, np.float32) + np.asarray(b_hh, np.float32)).copy()
    g_sl = slice(2 * H, 3 * H)
    Wih[g_sl] *= 2.0
    Whh[g_sl] *= 2.0
    bb[g_sl] *= 2.0

    def tiles(W):
        out = np.zeros((P, NJ * NK * P), np.float32)
        for jj in range(NJ):
            j = JPERM[jj]
            for k in range(NK):
                blk = W[j * P:(j + 1) * P, k * P:(k + 1) * P].T  # [K,M]
                out[:, (jj * NK + k) * P:(jj * NK + k + 1) * P] = blk
        return out.astype(bf16)

    whh_a = tiles(Whh)
    wih_a = tiles(Wih)
    Wout = np.asarray(W_out, np.float32)
    wout_a = np.zeros((P, NK * NT), np.float32)
    for k in range(NK):
        wout_a[:, k * NT:(k + 1) * NT] = Wout[:, k * P:(k + 1) * P].T
    wout_a = wout_a.astype(bf16)
    gbias_a = bb.reshape(NJ, P)[JPERM].T.copy().astype(np.float32)

    trans = np.asarray(transitions, np.float32)
    b_o = np.asarray(b_out, np.float32)
    transp = trans + b_o[:, None]
    finit = np.full((NT,), NEG, np.float32)
    finit[START] = 0.0
    estop = np.full((NT,), NEG, np.float32)
    estop[STOP] = 0.0
    ident_mp = np.full((NT, NT), NEG, np.float32)
    np.fill_diagonal(ident_mp, 0.0)
    vc = np.zeros((BL, 132), np.float32)
    vc[:, 0:36] = transp.reshape(-1)[None, :]
    vc[:, 36:72] = transp.T.reshape(-1)[None, :]
    vc[:, 72:78] = trans[STOP][None, :]
    vc[:, 78:84] = finit[None, :]
    vc[:, 84:90] = estop[None, :]
    vc[:, 90:96] = np.arange(NT, dtype=np.float32)[None, :]
    vc[:, 96:132] = ident_mp.reshape(-1)[None, :]

    vc2 = np.zeros((NCH * BL, 78), np.float32)
    vc2[:, 0:36] = transp.T.reshape(-1)[None, :]
    vc2[:, 36:72] = ident_mp.reshape(-1)[None, :]
    vc2[:, 72:78] = np.arange(NT, dtype=np.float32)[None, :]

    emb_a = np.ascontiguousarray(np.asarray(embedding, np.float32))
    return dict(emb=emb_a, whh=whh_a, wih=wih_a, wout=wout_a,
                gbias=gbias_a, vconst=vc, vconst2=vc2)


def make_in_maps(sentence, shared, t_steps=T):
    sent = np.asarray(sentence)
    in_maps = []
    for c in range(NCORES):
        loc = sent[c * BL:(c + 1) * BL, :t_steps]          # [BL, t]
        flat = loc.T.reshape(-1).astype(np.int32)          # (t,b)-major
        ng = (t_steps * BL) // P
        idx_a = flat.reshape(ng, P).T.copy()
        m = dict(shared)
        m["idx"] = np.ascontiguousarray(idx_a)
        in_maps.append(m)
    return in_maps


def kernel(sentence, embedding, W_ih, W_hh, b_ih, b_hh, W_out, b_out,
           transitions):
    from concourse.bass_utils import run_bass_kernel_spmd

    if "nc" not in _CACHE:
        _CACHE["nc"] = build_nc(T)
    nc = _CACHE["nc"]

    shared = prep_shared(embedding, W_ih, W_hh, b_ih, b_hh, W_out, b_out,
                         transitions)
    in_maps = make_in_maps(sentence, shared)
    res = run_bass_kernel_spmd(nc, in_maps, core_ids=list(range(NCORES)))

    scores = np.zeros((B,), np.float32)
    pdtype = np.int64 if np.asarray(sentence).dtype == np.int64 else np.int32
    paths = np.zeros((B, T), pdtype)
    for c in range(NCORES):
        scores[c * BL:(c + 1) * BL] = res.results[c]["scores"][:, 0]
        paths[c * BL:(c + 1) * BL] = res.results[c]["paths"].astype(pdtype)
    return scores, paths


# revision 27
# speedup vs baseline: 1.2377x; 1.0827x over previous
"""Trainium2 Bass kernel for BiLSTM-CRF (LSTM + CRF Viterbi decode).

Data-parallel over batch: 16 sequences sharded 2-per-core across 8 NeuronCores.
Per core: embedding gather (indirect DMA) -> input-projection GEMM (bf16 PE) ->
512-step LSTM with fused forward Viterbi scan -> chunk-parallel max-plus
backward scan -> bulk path extraction via argmax(mx_t + u_t).

Gate tiles are reordered (g,i | f,o) and the sigmoid is split in two so the
activation of the first half overlaps the second half's matmuls. tanh(g) is
computed as 2*sigmoid(2g)-1 with the g-rows of the weights pre-scaled by 2.
"""
import numpy as np
import ml_dtypes

VOCAB, H, B, T = 32000, 256, 16, 512
NT, START, STOP = 6, 4, 5
NEG = -10000.0
NCORES, BL = 8, 2
P = 128
NJ = 8          # 4H / 128 output tiles
NJH = 4         # tiles per half
NK = 2          # H / 128 contraction chunks
NCH = 32        # backward-scan chunks (batched over NCH*BL partitions)
# host tile order: g0 g1 i0 i1 | f0 f1 o0 o1  (original 4H tile index)
JPERM = [4, 5, 0, 1, 2, 3, 6, 7]

_CACHE = {}


def build_nc(t_steps=T, debug_dump=False):
    import concourse.bass as bass
    import concourse.tile as tile
    from concourse import bacc, mybir
    from concourse.masks import make_identity

    f32 = mybir.dt.float32
    bf16 = mybir.dt.bfloat16
    i32 = mybir.dt.int32
    ADD = mybir.AluOpType.add
    MULT = mybir.AluOpType.mult
    MAX = mybir.AluOpType.max
    MIN = mybir.AluOpType.min
    ISLT = mybir.AluOpType.is_lt
    SUB = mybir.AluOpType.subtract
    AX = mybir.AxisListType.X
    SIG = mybir.ActivationFunctionType.Sigmoid
    TANH = mybir.ActivationFunctionType.Tanh

    TB = t_steps * BL
    ng = TB // P
    CL = t_steps // NCH          # chunk length for backward scan
    PB = NCH * BL                # 16 partitions for batched backward

    nc = bacc.Bacc("TRN2", target_bir_lowering=False, debug=False,
                   num_devices=NCORES)

    emb = nc.dram_tensor("emb", [VOCAB, H], f32, kind="ExternalInput").ap()
    idx = nc.dram_tensor("idx", [P, ng], i32, kind="ExternalInput").ap()
    whh = nc.dram_tensor("whh", [P, NJ * NK * P], bf16, kind="ExternalInput").ap()
    wih = nc.dram_tensor("wih", [P, NJ * NK * P], bf16, kind="ExternalInput").ap()
    wout = nc.dram_tensor("wout", [P, NK * NT], bf16, kind="ExternalInput").ap()
    gbias = nc.dram_tensor("gbias", [P, NJ], f32, kind="ExternalInput").ap()
    vconst = nc.dram_tensor("vconst", [BL, 132], f32, kind="ExternalInput").ap()
    # cols: 0:36 transp 36:72 transpT 72:78 stop 78:84 finit 84:90 estop
    #       90:96 iota6 96:132 maxplus identity
    vconst2 = nc.dram_tensor("vconst2", [NCH * BL, 78], f32,
                             kind="ExternalInput").ap()
    # cols: 0:36 transpT 36:72 maxplus identity (j-major) 72:78 iota6
    scores_o = nc.dram_tensor("scores", [BL, 1], f32, kind="ExternalOutput").ap()
    if debug_dump:
        dbg_fb = nc.dram_tensor("dbg_fb", [NCH * BL, (t_steps // NCH) * NT], f32,
                                kind="ExternalOutput").ap()
        dbg_mx = nc.dram_tensor("dbg_mx", [NCH * BL, (t_steps // NCH) * NT], f32,
                                kind="ExternalOutput").ap()
        dbg_us = nc.dram_tensor("dbg_us", [NCH * BL, (t_steps // NCH) * NT], f32,
                                kind="ExternalOutput").ap()
        dbg_wv = nc.dram_tensor("dbg_wv", [BL, (NCH + 1) * NT], f32,
                                kind="ExternalOutput").ap()
        dbg_fbflat = nc.dram_tensor("dbg_fbflat", [BL, t_steps * NT], f32,
                                    kind="ExternalOutput").ap()
    paths_o = nc.dram_tensor("paths", [BL, t_steps], i32, kind="ExternalOutput").ap()

    with tile.TileContext(nc) as tc:
        with tc.tile_pool(name="const", bufs=1) as cpool, \
             tc.tile_pool(name="state", bufs=1) as spool, \
             tc.tile_pool(name="work", bufs=3) as wpool, \
             tc.tile_pool(name="hc", bufs=4) as hcpool, \
             tc.tile_pool(name="pga", bufs=2, space="PSUM") as pgapool, \
             tc.tile_pool(name="pgb", bufs=2, space="PSUM") as pgbpool, \
             tc.tile_pool(name="pf", bufs=2, space="PSUM") as pfpool:

            # ---- load constants / weights ----
            idx_t = cpool.tile([P, ng], i32, tag="idx")
            nc.sync.dma_start(idx_t[:], idx)
            whh_t = cpool.tile([P, NJ * NK * P], bf16, tag="whh")
            nc.sync.dma_start(whh_t[:], whh)
            wih_t = cpool.tile([P, NJ * NK * P], bf16, tag="wih")
            nc.sync.dma_start(wih_t[:], wih)
            wout_t = cpool.tile([P, NK * NT], bf16, tag="wout")
            nc.sync.dma_start(wout_t[:], wout)
            gb_t = cpool.tile([P, NJ], f32, tag="gb")
            nc.sync.dma_start(gb_t[:], gbias)
            vc_t = cpool.tile([BL, 132], f32, tag="vc")
            nc.sync.dma_start(vc_t[:], vconst)
            vc2_t = cpool.tile([NCH * BL, 78], f32, tag="vc2")
            nc.sync.dma_start(vc2_t[:], vconst2)
            ident = cpool.tile([P, P], bf16, tag="ident")
            make_identity(nc, ident[:])
            zero6 = cpool.tile([P, NT], bf16, tag="zero6")
            nc.gpsimd.memset(zero6[:], 0.0)

            # persistent buffers
            gx_t = cpool.tile([P, NJ * TB], f32, tag="gx")      # gates_x^T (j', t, b)
            xT_t = cpool.tile([P, NK * TB], bf16, tag="xT")     # x^T (k, t, b)
            fs_t = cpool.tile([BL, (t_steps + 1) * NT], f32, tag="fs")
            fbB_t = cpool.tile([BL, t_steps * NT], f32, tag="fbB")   # feats
            mxB_t = cpool.tile([BL, t_steps * NT], f32, tag="mxB")   # mx_t
            usB_t = cpool.tile([BL, t_steps * NT], f32, tag="usB")   # u_t

            # ---- gather + cast + transpose ----
            with tc.tile_pool(name="gath", bufs=3) as gpool, \
                 tc.tile_pool(name="ptr", bufs=2, space="PSUM") as ptpool:
                for g in range(ng):
                    xg = gpool.tile([P, H], f32, tag="xg")
                    nc.gpsimd.indirect_dma_start(
                        out=xg[:], out_offset=None, in_=emb,
                        in_offset=bass.IndirectOffsetOnAxis(ap=idx_t[:, g:g + 1], axis=0),
                    )
                    xb = gpool.tile([P, H], bf16, tag="xb")
                    nc.vector.tensor_copy(xb[:], xg[:])
                    for k in range(NK):
                        pt = ptpool.tile([P, P], bf16, space="PSUM", tag="pt")
                        nc.tensor.transpose(out=pt[:], in_=xb[:, k * P:(k + 1) * P],
                                            identity=ident[:])
                        nc.vector.tensor_copy(xT_t[:, k * TB + g * P:k * TB + (g + 1) * P],
                                              pt[:])

            # ---- bulk input-projection GEMM: gx[j', tb] = sum_k wih[j',k].T @ xT[k] ----
            NHALF = max(1, TB // 512)
            HW_N = min(TB, 512)
            with tc.tile_pool(name="pb", bufs=2, space="PSUM") as pbpool:
                for j in range(NJ):
                    for hh in range(NHALF):
                        pb = pbpool.tile([P, HW_N], f32, space="PSUM", tag="pb")
                        for k in range(NK):
                            nc.tensor.matmul(
                                out=pb[:],
                                lhsT=wih_t[:, (j * NK + k) * P:(j * NK + k + 1) * P],
                                rhs=xT_t[:, k * TB + hh * HW_N:k * TB + (hh + 1) * HW_N],
                                start=(k == 0), stop=(k == NK - 1))
                        nc.vector.tensor_tensor(
                            out=gx_t[:, j * TB + hh * HW_N:j * TB + (hh + 1) * HW_N],
                            in0=pb[:], in1=gb_t[:, j:j + 1].to_broadcast([P, HW_N]),
                            op=ADD)

            # ---- init state ----
            h0_prev = hcpool.tile([P, BL], bf16, tag="h0")
            nc.gpsimd.memset(h0_prev[:], 0.0)
            h1_prev = hcpool.tile([P, BL], bf16, tag="h1")
            nc.gpsimd.memset(h1_prev[:], 0.0)
            c_prev = hcpool.tile([P, NK * BL], f32, tag="c")
            nc.gpsimd.memset(c_prev[:], 0.0)
            nc.vector.tensor_copy(fs_t[:, 0:NT], vc_t[:, 78:84])   # init fv

            gxA = gx_t[:].rearrange("p (j tb) -> p j tb", j=NJ)
            trans_f = vc_t[:, 0:36].rearrange("q (n m) -> q n m", n=NT)

            def feat_and_viterbi(t, hl0, hl1, gate0=None):
                """Emission matmuls + forward-viterbi ops for step t.

                gate0: tile whose readiness gates the PE group (zero-matmul
                trick) so emission matmuls don't interleave with gate matmuls.
                """
                pf = pfpool.tile([BL, NT], f32, space="PSUM", tag="pf")
                if gate0 is not None:
                    nc.tensor.matmul(out=pf[:], lhsT=gate0[:, 0:BL],
                                     rhs=zero6[:], start=True, stop=False)
                for k, hl in ((0, hl0), (1, hl1)):
                    nc.tensor.matmul(out=pf[:], lhsT=hl[:],
                                     rhs=wout_t[:, k * NT:(k + 1) * NT],
                                     start=(gate0 is None and k == 0),
                                     stop=(k == NK - 1))
                sc = wpool.tile([BL, NT * NT], f32, tag="sc")
                nc.gpsimd.tensor_tensor(
                    out=sc[:].rearrange("q (n m) -> q n m", n=NT),
                    in0=fs_t[:, t * NT:(t + 1) * NT]
                        .rearrange("q (a m) -> q a m", a=1).broadcast_to([BL, NT, NT]),
                    in1=trans_f, op=ADD)
                nc.vector.tensor_reduce(
                    out=mxB_t[:, t * NT:(t + 1) * NT],
                    in_=sc[:].rearrange("q (n m) -> q n m", n=NT), axis=AX, op=MAX)
                nc.vector.tensor_scalar_add(fbB_t[:, t * NT:(t + 1) * NT],
                                            pf[:], 0.0)
                nc.gpsimd.tensor_tensor(
                    out=fs_t[:, (t + 1) * NT:(t + 2) * NT],
                    in0=mxB_t[:, t * NT:(t + 1) * NT],
                    in1=fbB_t[:, t * NT:(t + 1) * NT], op=ADD)

            # ---- main LSTM loop ----
            for t in range(t_steps):
                pga = pgapool.tile([P, NJH * BL], f32, space="PSUM", tag="pga")
                pgb = pgbpool.tile([P, NJH * BL], f32, space="PSUM", tag="pgb")
                for half, pg in ((0, pga), (1, pgb)):
                    for jj in range(NJH):
                        j = half * NJH + jj
                        for k, hl in ((0, h0_prev), (1, h1_prev)):
                            nc.tensor.matmul(
                                out=pg[:, jj * BL:(jj + 1) * BL],
                                lhsT=whh_t[:, (j * NK + k) * P:(j * NK + k + 1) * P],
                                rhs=hl[:], start=(k == 0), stop=(k == NK - 1))
                gsA = wpool.tile([P, NJH * BL], f32, tag="gsA")
                nc.vector.tensor_tensor(
                    out=gsA[:].rearrange("p (j b) -> p j b", j=NJH),
                    in0=pga[:].rearrange("p (j b) -> p j b", j=NJH),
                    in1=gxA[:, 0:NJH, t * BL:(t + 1) * BL], op=ADD)
                gaA = wpool.tile([P, NJH * BL], f32, tag="gaA")   # g0 g1 i0 i1
                nc.scalar.activation(gaA[:], gsA[:], SIG)
                g2 = wpool.tile([P, NK * BL], f32, tag="g2")
                nc.vector.tensor_scalar(out=g2[:], in0=gaA[:, 0:4], scalar1=2.0,
                                        scalar2=-1.0, op0=MULT, op1=ADD)
                ig = wpool.tile([P, NK * BL], f32, tag="ig")
                nc.vector.tensor_tensor(out=ig[:], in0=gaA[:, 4:8], in1=g2[:], op=MULT)

                gsB = wpool.tile([P, NJH * BL], f32, tag="gsB")
                nc.vector.tensor_tensor(
                    out=gsB[:].rearrange("p (j b) -> p j b", j=NJH),
                    in0=pgb[:].rearrange("p (j b) -> p j b", j=NJH),
                    in1=gxA[:, NJH:NJ, t * BL:(t + 1) * BL], op=ADD)
                gaB = wpool.tile([P, NJH * BL], f32, tag="gaB")   # f0 f1 o0 o1
                nc.scalar.activation(gaB[:], gsB[:], SIG)
                fc = wpool.tile([P, NK * BL], f32, tag="fc")
                nc.vector.tensor_tensor(out=fc[:], in0=gaB[:, 0:4], in1=c_prev[:],
                                        op=MULT)
                c_new = hcpool.tile([P, NK * BL], f32, tag="c")
                nc.vector.tensor_tensor(out=c_new[:], in0=fc[:], in1=ig[:], op=ADD)
                th = wpool.tile([P, NK * BL], f32, tag="th")
                nc.scalar.activation(th[:], c_new[:], TANH)
                h0_new = hcpool.tile([P, BL], bf16, tag="h0")
                nc.vector.tensor_tensor(out=h0_new[:], in0=gaB[:, 4:6],
                                        in1=th[:, 0:2], op=MULT)
                h1_new = hcpool.tile([P, BL], bf16, tag="h1")
                nc.vector.tensor_tensor(out=h1_new[:], in0=gaB[:, 6:8],
                                        in1=th[:, 2:4], op=MULT)
                if t > 0:
                    with tc.high_priority(offset=-50):
                        feat_and_viterbi(t - 1, h0_prev, h1_prev, gate0=h0_new)
                h0_prev, h1_prev, c_prev = h0_new, h1_new, c_new

            feat_and_viterbi(t_steps - 1, h0_prev, h1_prev)

            # ---- backward: chunk-parallel max-plus scan over u_t = b_t + feat_t ----
            # batched over 16 partitions (row = b*NCH + c) via SBUF->SBUF
            # reshape DMAs; M stored j-major: M2[j, r] = M[r, j]
            PB = NCH * BL
            fbB16 = spool.tile([PB, CL * NT], f32, tag="fbB16")
            mxB16 = spool.tile([PB, CL * NT], f32, tag="mxB16")
            for b in range(BL):
                nc.sync.dma_start(fbB16[b * NCH:(b + 1) * NCH, :],
                                  fbB_t[b:b + 1, :])
                nc.sync.dma_start(mxB16[b * NCH:(b + 1) * NCH, :],
                                  mxB_t[b:b + 1, :])
            transT16 = vc2_t[:, 0:36]
            Mt = spool.tile([PB, 36], f32, tag="Mt16")
            nc.vector.tensor_copy(Mt[:], vc2_t[:, 36:72])
            for l in range(CL - 1, -1, -1):
                tmp = wpool.tile([PB, 216], f32, tag="btmp")
                mt_ap = Mt[:]
                nc.vector.tensor_tensor(
                    out=tmp[:].rearrange("p (j m k) -> p j m k", j=NT, m=NT),
                    in0=bass.AP(tensor=mt_ap.tensor, offset=mt_ap.offset,
                                ap=[mt_ap.ap[0], [NT, NT], [0, NT], [1, NT]]),
                    in1=bass.AP(tensor=transT16.tensor, offset=transT16.offset,
                                ap=[transT16.ap[0], [0, NT], [NT, NT], [1, NT]]),
                    op=ADD)
                red = wpool.tile([PB, 36], f32, tag="bred")
                nc.vector.tensor_reduce(
                    out=red[:],
                    in_=tmp[:].rearrange("p (j m k) -> p j m k", j=NT, m=NT),
                    axis=AX, op=MAX)
                Mt_new = spool.tile([PB, 36], f32, tag="Mt16b")
                nc.vector.tensor_tensor(
                    out=Mt_new[:].rearrange("p (j m) -> p j m", j=NT),
                    in0=red[:].rearrange("p (j m) -> p j m", j=NT),
                    in1=fbB16[:, l * NT:(l + 1) * NT]
                        .rearrange("p (a m) -> p a m", a=1)
                        .broadcast_to([PB, NT, NT]),
                    op=ADD)
                Mt = Mt_new
            # bring M back to [BL, (c, j, m)] for the sequential boundary pass
            MtF = spool.tile([BL, NCH * 36], f32, tag="MtF")
            for b in range(BL):
                nc.sync.dma_start(MtF[b:b + 1, :], Mt[b * NCH:(b + 1) * NCH, :])
            # boundary combine: w_c = N_c (x) w_{c+1}, w_NCH = e_stop
            wv = spool.tile([BL, (NCH + 1) * NT], f32, tag="wv")
            nc.vector.tensor_copy(wv[:, NCH * NT:(NCH + 1) * NT], vc_t[:, 84:90])
            for c_i in range(NCH - 1, -1, -1):
                t1 = wpool.tile([BL, 36], f32, tag="bt1")
                nc.vector.tensor_tensor(
                    out=t1[:].rearrange("q (m j) -> q m j", m=NT),
                    in0=wv[:, (c_i + 1) * NT:(c_i + 2) * NT]
                        .rearrange("q (a j) -> q a j", a=1).broadcast_to([BL, NT, NT]),
                    in1=MtF[:, c_i * 36:(c_i + 1) * 36]
                        .rearrange("q (j m) -> q m j", j=NT),
                    op=ADD)
                nc.vector.tensor_reduce(
                    out=wv[:, c_i * NT:(c_i + 1) * NT],
                    in_=t1[:].rearrange("q (m j) -> q m j", m=NT), axis=AX, op=MAX)
            # seed u at chunk ends: ucur[c] = w_{c+1}
            useed2 = spool.tile([BL, NCH * NT], f32, tag="useed2")
            nc.vector.tensor_scalar_add(useed2[:], wv[:, NT:(NCH + 1) * NT], 0.0)
            useed16 = spool.tile([PB, NT], f32, tag="useed16")
            for b in range(BL):
                nc.sync.dma_start(useed16[b * NCH:(b + 1) * NCH, :],
                                  useed2[b:b + 1, :])
            usB16 = spool.tile([PB, CL * NT], f32, tag="usB16")
            ucur_ap = useed16[:]
            # vector pass: u_l = A'_l (x) u_{l+1}
            for l in range(CL - 1, -1, -1):
                tmp2 = wpool.tile([PB, 36], f32, tag="vtmp")
                nc.vector.tensor_tensor(
                    out=tmp2[:].rearrange("p (m k) -> p m k", m=NT),
                    in0=ucur_ap.rearrange("p (a k) -> p a k", a=1)
                        .broadcast_to([PB, NT, NT]),
                    in1=transT16.rearrange("p (m k) -> p m k", m=NT), op=ADD)
                red2 = wpool.tile([PB, NT], f32, tag="vred")
                nc.vector.tensor_reduce(
                    out=red2[:], in_=tmp2[:].rearrange("p (m k) -> p m k", m=NT),
                    axis=AX, op=MAX)
                nc.vector.tensor_tensor(
                    out=usB16[:, l * NT:(l + 1) * NT], in0=red2[:],
                    in1=fbB16[:, l * NT:(l + 1) * NT], op=ADD)
                ucur_ap = usB16[:, l * NT:(l + 1) * NT]

            # ---- bulk path extraction: path[t] = argmax_n(mx_t + u_t) ----
            ps = spool.tile([PB, CL * NT], f32, tag="ps")
            nc.vector.tensor_tensor(out=ps[:], in0=mxB16[:], in1=usB16[:], op=ADD)
            ps3 = ps[:].rearrange("q (t n) -> q t n", n=NT)
            mxp = spool.tile([PB, CL], f32, tag="mxp")
            nc.vector.tensor_reduce(out=mxp[:], in_=ps3, axis=AX, op=MAX)
            lt = spool.tile([PB, CL * NT], f32, tag="lt")
            nc.vector.tensor_tensor(
                out=lt[:].rearrange("q (t n) -> q t n", n=NT), in0=ps3,
                in1=mxp[:].rearrange("q (t a) -> q t a", a=1)
                    .broadcast_to([PB, CL, NT]),
                op=ISLT)
            val = spool.tile([PB, CL * NT], f32, tag="val")
            nc.vector.scalar_tensor_tensor(
                out=val[:].rearrange("q (t n) -> q t n", n=NT),
                in0=lt[:].rearrange("q (t n) -> q t n", n=NT),
                scalar=1024.0,
                in1=vc2_t[:, 72:78].rearrange("q (a n) -> q a n", a=1)
                    .broadcast_to([PB, CL, NT]),
                op0=MULT, op1=ADD)
            pidx = spool.tile([PB, CL], f32, tag="pidx")
            nc.vector.tensor_reduce(out=pidx[:],
                                    in_=val[:].rearrange("q (t n) -> q t n", n=NT),
                                    axis=AX, op=MIN)
            pi32 = spool.tile([PB, CL], i32, tag="pi32")
            nc.vector.tensor_copy(pi32[:], pidx[:])
            nc.sync.dma_start(
                paths_o.rearrange("b (c l) -> b c l", c=NCH), pi32[:])

            if debug_dump:
                dbgc = spool.tile([BL, t_steps * NT], f32, tag="dbgc")
                nc.vector.tensor_scalar_add(dbgc[:], fbB_t[:], 1000.0)
                nc.sync.dma_start(dbg_fbflat, dbgc[:])
                nc.sync.dma_start(dbg_fb, fbB16[:])
                nc.sync.dma_start(dbg_mx, mxB16[:])
                nc.sync.dma_start(dbg_us, usB16[:])
                nc.sync.dma_start(dbg_wv, wv[:])
            term = spool.tile([BL, NT], f32, tag="term")
            nc.vector.tensor_tensor(out=term[:],
                                    in0=fs_t[:, t_steps * NT:(t_steps + 1) * NT],
                                    in1=vc_t[:, 72:78], op=ADD)
            scr = spool.tile([BL, 1], f32, tag="scr")
            nc.vector.tensor_reduce(out=scr[:], in_=term[:], axis=AX, op=MAX)
            nc.sync.dma_start(scores_o, scr[:])

    nc.compile()
    return nc


def prep_shared(embedding, W_ih, W_hh, b_ih, b_hh, W_out, b_out, transitions):
    """Host-side weight prep shared across cores."""
    bf16 = ml_dtypes.bfloat16
    Wih = np.asarray(W_ih, np.float32).copy()
    Whh = np.asarray(W_hh, np.float32).copy()
    bb = (np.asarray(b_ih, np.float32) + np.asarray(b_hh, np.float32)).copy()
    g_sl = slice(2 * H, 3 * H)
    Wih[g_sl] *= 2.0
    Whh[g_sl] *= 2.0
    bb[g_sl] *= 2.0

    def tiles(W):
        out = np.zeros((P, NJ * NK * P), np.float32)
        for jj in range(NJ):
            j = JPERM[jj]
            for k in range(NK):
                blk = W[j * P:(j + 1) * P, k * P:(k + 1) * P].T  # [K,M]
                out[:, (jj * NK + k) * P:(jj * NK + k + 1) * P] = blk
        return out.astype(bf16)

    whh_a = tiles(Whh)
    wih_a = tiles(Wih)
    Wout = np.asarray(W_out, np.float32)
    wout_a = np.zeros((P, NK * NT), np.float32)
    for k in range(NK):
        wout_a[:, k * NT:(k + 1) * NT] = Wout[:, k * P:(k + 1) * P].T
    wout_a = wout_a.astype(bf16)
    gbias_a = bb.reshape(NJ, P)[JPERM].T.copy().astype(np.float32)

    trans = np.asarray(transitions, np.float32)
    b_o = np.asarray(b_out, np.float32)
    transp = trans + b_o[:, None]
    finit = np.full((NT,), NEG, np.float32)
    finit[START] = 0.0
    estop = np.full((NT,), NEG, np.float32)
    estop[STOP] = 0.0
    ident_mp = np.full((NT, NT), NEG, np.float32)
    np.fill_diagonal(ident_mp, 0.0)
    vc = np.zeros((BL, 132), np.float32)
    vc[:, 0:36] = transp.reshape(-1)[None, :]
    vc[:, 36:72] = transp.T.reshape(-1)[None, :]
    vc[:, 72:78] = trans[STOP][None, :]
    vc[:, 78:84] = finit[None, :]
    vc[:, 84:90] = estop[None, :]
    vc[:, 90:96] = np.arange(NT, dtype=np.float32)[None, :]
    vc[:, 96:132] = ident_mp.reshape(-1)[None, :]

    vc2 = np.zeros((NCH * BL, 78), np.float32)
    vc2[:, 0:36] = transp.T.reshape(-1)[None, :]
    vc2[:, 36:72] = ident_mp.reshape(-1)[None, :]
    vc2[:, 72:78] = np.arange(NT, dtype=np.float32)[None, :]

    emb_a = np.ascontiguousarray(np.asarray(embedding, np.float32))
    return dict(emb=emb_a, whh=whh_a, wih=wih_a, wout=wout_a,
                gbias=gbias_a, vconst=vc, vconst2=vc2)


def make_in_maps(sentence, shared, t_steps=T):
    sent = np.asarray(sentence)
    in_maps = []
    for c in range(NCORES):
        loc = sent[c * BL:(c + 1) * BL, :t_steps]          # [BL, t]
        flat = loc.T.reshape(-1).astype(np.int32)          # (t,b)-major
        ng = (t_steps * BL) // P
        idx_a = flat.reshape(ng, P).T.copy()
        m = dict(shared)
        m["idx"] = np.ascontiguousarray(idx_a)
        in_maps.append(m)
    return in_maps


def kernel(sentence, embedding, W_ih, W_hh, b_ih, b_hh, W_out, b_out,
           transitions):
    from concourse.bass_utils import run_bass_kernel_spmd

    if "nc" not in _CACHE:
        _CACHE["nc"] = build_nc(T)
    nc = _CACHE["nc"]

    shared = prep_shared(embedding, W_ih, W_hh, b_ih, b_hh, W_out, b_out,
                         transitions)
    in_maps = make_in_maps(sentence, shared)
    res = run_bass_kernel_spmd(nc, in_maps, core_ids=list(range(NCORES)))

    scores = np.zeros((B,), np.float32)
    pdtype = np.int64 if np.asarray(sentence).dtype == np.int64 else np.int32
    paths = np.zeros((B, T), pdtype)
    for c in range(NCORES):
        scores[c * BL:(c + 1) * BL] = res.results[c]["scores"][:, 0]
        paths[c * BL:(c + 1) * BL] = res.results[c]["paths"].astype(pdtype)
    return scores, paths
